# revision 11
# baseline (speedup 1.0000x reference)
"""Trainium2 Bass kernel for nn_BatchTrainableButterfly (v4.2).

The reference applies, per mesh-batch b, a trainable butterfly network
(10 levels of phase shifters + 2x2 directional couplers with butterfly
permutations, plus a final phase layer and bit-reversals) to every token
row x[n, :].  For fixed phases the network is linear on C^1024 and
factorizes into two block stages:

  Stage A = input bitrev + levels 0..6: 8 independent dense 128x128
  complex blocks; block g consumes x columns {8p + rev3(g)}.
  Stage B = butterfly perm + levels 7..9 + final phase + final bitrev +
  scale: per-position 8x8 mixing across the 8 blocks, extracted as 8
  dense 128x128 complex matrices (t2-groups of 16 positions each).

Layout: token-sharded SPMD — each of the 8 cores takes 512 tokens and
runs all 4 mesh-batches.  Everything moves in bf16 (host casts both
ways; rel-err budget 2e-2).  x reaches the device already transposed
(and pre-negated for the imaginary stream) by the host — no device
input transposes.  Stage B runs "reversed" — the shuffled stage-A
output tiles are the PE stationary, the B matrices are the moving
operand — so the output comes out token-major with no output transposes
either; its columns are stored t2-grouped (contiguous PSUM->SBUF
copies) and the host applies the final position permutation.  The only
PE work is real matmuls; the only inter-stage data motion is the
unavoidable 128-partition corner-turn, done as 8 SBUF->SBUF DMAs per
batch.  Inputs arrive in a handful of large DMAs to keep the head of
the kernel short.
"""

import math

import numpy as np

import concourse.tile as tile
from concourse import bacc, bass, mybir
from concourse.bass_utils import run_bass_kernel_spmd

P = 128          # partitions
L = 1024         # butterfly length
N_TOKENS = 4096
MESH_BATCH = 4
N_CORES = 8
TC = N_TOKENS // N_CORES   # 512 tokens per core
NTT = TC // P              # 4 token tiles per core
NLEV = int(math.log2(L))   # 10

F32 = mybir.dt.float32
BF16 = mybir.dt.bfloat16

N_WARM = 16      # dummy matmuls to lift the PE HAM clock gate while DMAs land

TRACE = False
LAST_RESULTS = None

# ----------------------------------------------------------------------
# Host side: two-stage factorization of the butterfly network.
# ----------------------------------------------------------------------


def _bitrev(n):
    m = int(math.log2(n))
    perm = np.arange(n).reshape(n, 1)
    for _ in range(m):
        n1 = perm.shape[0] // 2
        perm = np.hstack((perm[:n1], perm[n1:]))
    return perm.squeeze(0)


def _forward_indices(length):
    idx = []
    ar = np.arange(length)
    for level in range(int(math.log2(length)) - 1):
        bs = 2 ** (level + 2)
        ind = ar.reshape(-1, length // bs, 2, bs // 2).transpose(0, 1, 3, 2)
        idx.append(ind.reshape(-1))
    return idx


def _rev(v, n):
    r = 0
    for _ in range(n):
        r = (r << 1) | (v & 1)
        v >>= 1
    return r


def _stage_matrices(phases):
    """Astat[b, r] (K=p x M=c): K is x index 8p+r, col c -> stage-A output
    partition c = s*8+t2 holding block position t2*16+s of block g=rev3(r).
    Bstat[b, t2] (K x C): K-row k = s*8+g sources block g position t2*16+s,
    col c -> final output position 128*(c%8) + 8*(c//8) + rev3(t2)."""
    B_ = phases.shape[0]
    br = _bitrev(L)
    fidx = _forward_indices(L)
    dc = np.array([[1.0, 1.0j], [1.0j, 1.0]], dtype=np.complex64)

    def levels(x, lo, hi, pre_br=False, post_final=False, pre_perm=None):
        if pre_br:
            x = x[..., br]
        if pre_perm is not None:
            x = x[..., pre_perm]
        for level in range(lo, hi):
            x = x.reshape(B_, L, L // 2, 2)
            x = x * np.exp(1j * phases[:, level, None, :, :].astype(np.complex64))
            x = x @ dc
            x = x.reshape(B_, L, L)
            if level < NLEV - 1 and level != 6:
                x = x[..., fidx[level]]
        if post_final:
            x = x * np.exp(
                1j * phases[:, NLEV - 1, None, :, :].reshape(B_, 1, L).astype(np.complex64)
            )
            x = x[..., br]
            x = x / np.float32(np.sqrt(L))
        return x

    eye = np.broadcast_to(np.eye(L, dtype=np.complex64), (B_, L, L)).copy()
    A = levels(eye.copy(), 0, 7, pre_br=True)
    Bm = levels(eye.copy(), 7, NLEV, post_final=True, pre_perm=fidx[6])

    ar_ = np.arange(P)
    posperm = (ar_ & 7) * 16 + (ar_ >> 3)
    Astat = np.empty((B_, 8, P, P), dtype=np.complex64)
    for r in range(8):
        g = _rev(r, 3)
        Astat[:, r] = A[:, ar_ * 8 + r][:, :, g * P + posperm]

    s_, g_ = np.divmod(ar_, 8)
    v_, m_ = np.divmod(ar_, 8)
    Bstat = np.empty((B_, 8, P, P), dtype=np.complex64)
    for t2 in range(8):
        rows = g_ * P + t2 * 16 + s_
        cols = P * m_ + 8 * v_ + _rev(t2, 3)
        Bstat[:, t2] = Bm[:, rows][:, :, cols]
    return Astat, Bstat


# ----------------------------------------------------------------------
# Device side.
# ----------------------------------------------------------------------

# mats tile column layout (in units of P columns):
#   [0:8]   ar   (8 r-blocks)
#   [8:16]  ai
#   [16:40] b-movers: per t2 a 3*P block [Br | Bi | nBi]
MAT_W = 40 * P


def _build_program():
    # detect_race_conditions=False: the rust race detector false-positives on
    # the stepped-partition shuffle DMA vs writes to a *different* bin buffer
    # (disjoint SBUF regions sharing a shadow zone). Same-tensor deps are
    # tracked normally.
    nc = bacc.Bacc(
        "TRN2", target_bir_lowering=False, debug=False, num_devices=N_CORES,
        detect_race_conditions=False,
    )

    # x pre-transposed on host: plane r*P+p holds x[:, 8p+r] for this core's
    # TC tokens. xre separate; (xi, nxi) stacked so each loads as one DMA.
    xre_d = nc.declare_dram_parameter("xre", [8 * P, TC], BF16, isOutput=False)
    xim_d = nc.declare_dram_parameter("xim", [2 * 8 * P, TC], BF16, isOutput=False)
    # All matrices for one batch in one row-block: [b*P+k, MAT_W]
    mat_d = nc.declare_dram_parameter("mat", [MESH_BATCH * P, MAT_W], BF16, isOutput=False)
    # Output, position-major: row (b*8+t2)*P + c holds (re tokens | im tokens);
    # host does the token-major transpose + position permutation.
    out_d = nc.declare_dram_parameter("out", [MESH_BATCH * 8 * P, 2 * TC], BF16, isOutput=True)

    with tile.TileContext(nc) as tc:
        with (
            tc.tile_pool(name="const", bufs=1) as const_pool,
            tc.tile_pool(name="mats", bufs=1) as mat_pool,
            tc.tile_pool(name="xt", bufs=1) as xt_pool,
            tc.tile_pool(name="ya", bufs=12) as ya_pool,
            tc.tile_pool(name="bin", bufs=1) as bin_pool,
            tc.tile_pool(name="osb", bufs=2) as o_pool,
            tc.tile_pool(name="psA", bufs=2, space=bass.MemorySpace.PSUM) as psA_pool,
            tc.tile_pool(name="psB", bufs=2, space=bass.MemorySpace.PSUM) as psB_pool,
        ):
            # Warmup operand (zeros so sim sees initialized reads).
            wz = const_pool.tile([P, TC], BF16)
            nc.gpsimd.memset(wz[:], 0.0)
            for i in range(N_WARM):
                warm = psA_pool.tile([P, 2, TC], F32, tag="psA", name=f"warm{i}")
                nc.tensor.matmul(warm[:, 0, :], wz[:, 0:P], wz[:], start=True, stop=True)

            # Shuffle destinations, double-buffered across batches; memset once
            # so the stepped-partition DMA writes are observable to sim init
            # tracking (overlaps warmup / input DMAs).
            bn_bufs = []
            for i in range(2):
                bnb = bin_pool.tile([P, 8, 2 * TC], BF16, tag=f"bin{i}")
                nc.gpsimd.memset(bnb[:], 0.0)
                bn_bufs.append(bnb)

            # Inputs: one DMA for xre, one for (xi, nxi), one per batch for mats.
            xre = xt_pool.tile([P, 8, TC], BF16)
            nc.sync.dma_start(
                out=xre[:],
                in_=xre_d[:, :].rearrange("(r p) t -> p r t", p=P),
            )
            xim = xt_pool.tile([P, 2, 8, TC], BF16)
            nc.scalar.dma_start(
                out=xim[:],
                in_=xim_d[:, :].rearrange("(n r p) t -> p n r t", p=P, n=2),
            )
            mats = {}
            for b in range(MESH_BATCH):
                t_ = mat_pool.tile([P, MAT_W], BF16, tag=f"mat{b}", name=f"mat{b}")
                nc.gpsimd.dma_start(out=t_[:], in_=mat_d[b * P : (b + 1) * P, :])
                mats[b] = t_

            def xT(pl, r):
                if pl == 0:
                    return xre[:, r, :]
                return xim[:, pl - 1, r, :]

            def emit_A(b):
                """Stage A + corner-turn shuffle for batch b."""
                bn = bn_bufs[b % 2]
                for r in range(8):
                    g = _rev(r, 3)
                    ars = mats[b][:, r * P : (r + 1) * P]
                    ais = mats[b][:, (8 + r) * P : (9 + r) * P]
                    pa = psA_pool.tile([P, 2, TC], F32, tag="psA", name=f"pa_{b}_{r}")
                    # grouped by stationary: 2 weight loads per block
                    nc.tensor.matmul(pa[:, 0, :], ars, xT(0, r), start=True, stop=False)
                    nc.tensor.matmul(pa[:, 1, :], ars, xT(1, r), start=True, stop=False)
                    nc.tensor.matmul(pa[:, 1, :], ais, xT(0, r), start=False, stop=True)
                    nc.tensor.matmul(pa[:, 0, :], ais, xT(2, r), start=False, stop=True)
                    ya = ya_pool.tile([P, 2 * TC], BF16, tag="ya", name=f"ya_{b}_{r}")
                    eng = nc.vector.tensor_copy if (r % 2) else nc.scalar.copy
                    eng(ya[:], pa[:])
                    # corner turn: bn[s*8+g, t2, :] = ya[s*8+t2, :]
                    deng = nc.scalar if (r % 2) else nc.sync
                    deng.dma_start(out=bn[g:P:8, :, :], in_=ya[:])
                return bn

            def emit_B(b, bn):
                """Stage B for batch b: stationary = B matrices (all weight
                loads hide under N=512 matmuls), moving = shuffled stage-A
                tiles -> position-major output; host transposes."""
                osb = o_pool.tile([P, 8, 2 * TC], BF16, tag="osb", name=f"osb{b}")
                for t2 in range(8):
                    base = (16 + 3 * t2) * P
                    brs = mats[b][:, base : base + P]
                    bis = mats[b][:, base + P : base + 2 * P]
                    nbis = mats[b][:, base + 2 * P : base + 3 * P]
                    bre = bn[:, t2, 0:TC]
                    bim = bn[:, t2, TC : 2 * TC]
                    pb = psB_pool.tile([P, 2, TC], F32, tag="psB", name=f"pb_{b}_{t2}")
                    # grouped by stationary: 3 weight loads per t2-group
                    nc.tensor.matmul(pb[:, 0, :], brs, bre, start=True, stop=False)
                    nc.tensor.matmul(pb[:, 1, :], brs, bim, start=True, stop=False)
                    nc.tensor.matmul(pb[:, 1, :], bis, bre, start=False, stop=True)
                    nc.tensor.matmul(pb[:, 0, :], nbis, bim, start=False, stop=True)
                    eng = nc.vector.tensor_copy if (t2 % 2) else nc.scalar.copy
                    eng(osb[:, t2, :], pb[:])
                # one 2MB store for the whole batch: row (b*8+t2)*P + c
                r0 = b * 8 * P
                deng = nc.scalar if (b % 2) else nc.sync
                deng.dma_start(
                    out=out_d[r0 : r0 + 8 * P, :].rearrange("(t c) f -> c t f", c=P),
                    in_=osb[:],
                )

            # Software pipeline across batches: stage B of batch b-1 is
            # emitted after stage A (and shuffle issue) of batch b.
            prev = None
            for b in range(MESH_BATCH):
                bn = emit_A(b)
                if prev is not None:
                    emit_B(prev[0], prev[1])
                prev = (b, bn)
            emit_B(prev[0], prev[1])

    nc.compile()
    return nc


_CACHED = {}


def kernel(x_re: np.ndarray, x_im: np.ndarray, phases: np.ndarray) -> np.ndarray:
    global LAST_RESULTS
    import ml_dtypes

    BF = ml_dtypes.bfloat16

    x_re = np.ascontiguousarray(x_re, dtype=np.float32)
    x_im = np.ascontiguousarray(x_im, dtype=np.float32)
    phases = np.ascontiguousarray(phases, dtype=np.float32)

    Astat, Bstat = _stage_matrices(phases)
    # Per-batch combined matrix block: [b, k, MAT_W]
    mat = np.empty((MESH_BATCH, P, 40, P), dtype=np.float32)
    mat[:, :, 0:8, :] = Astat.real.transpose(0, 2, 1, 3)
    mat[:, :, 8:16, :] = Astat.imag.transpose(0, 2, 1, 3)
    Bre = Bstat.real.transpose(0, 2, 1, 3)     # [b, k, t2, c]
    Bim = Bstat.imag.transpose(0, 2, 1, 3)
    for t2 in range(8):
        mat[:, :, 16 + 3 * t2, :] = Bre[:, :, t2, :]
        mat[:, :, 17 + 3 * t2, :] = Bim[:, :, t2, :]
        mat[:, :, 18 + 3 * t2, :] = -Bim[:, :, t2, :]
    mat = np.ascontiguousarray(mat.reshape(MESH_BATCH * P, MAT_W)).astype(BF)

    # Host-side input transpose: xt[r, p, tok] = x[tok, 8p+r], bf16.
    xrt = np.ascontiguousarray(
        x_re.astype(BF).reshape(N_TOKENS, P, 8).transpose(2, 1, 0)
    )  # (8, 128, N)
    xit = np.ascontiguousarray(
        x_im.astype(BF).reshape(N_TOKENS, P, 8).transpose(2, 1, 0)
    )
    nxit = np.ascontiguousarray(
        (-x_im).astype(BF).reshape(N_TOKENS, P, 8).transpose(2, 1, 0)
    )

    if "v4" not in _CACHED:
        _CACHED["v4"] = _build_program()
    nc = _CACHED["v4"]

    in_maps = []
    for c in range(N_CORES):
        tok = slice(c * TC, (c + 1) * TC)
        xim_stack = np.empty((2, 8, P, TC), dtype=BF)
        xim_stack[0] = xit[:, :, tok]
        xim_stack[1] = nxit[:, :, tok]
        in_maps.append(
            {
                "xre": np.ascontiguousarray(xrt[:, :, tok]).reshape(8 * P, TC),
                "xim": xim_stack.reshape(16 * P, TC),
                "mat": mat,
            }
        )

    res = run_bass_kernel_spmd(nc, in_maps, list(range(N_CORES)), trace=TRACE)
    LAST_RESULTS = res

    # Final column permutation: device col (t2, comp, c) -> position
    # j = 128*(c%8) + 8*(c//8) + rev3(t2).
    c_ = np.arange(P)
    jidx = np.empty((8, P), dtype=np.int64)
    for t2 in range(8):
        jidx[t2] = P * (c_ % 8) + 8 * (c_ // 8) + _rev(t2, 3)
    jflat = jidx.reshape(8 * P)
    inv = np.empty_like(jflat)
    inv[jflat] = np.arange(8 * P)

    out = np.empty((MESH_BATCH, N_TOKENS, L), dtype=np.complex64)
    for c in range(N_CORES):
        buf = np.asarray(res.results[c]["out"]).astype(np.float32)
        z = buf.reshape(MESH_BATCH, 8, P, 2, TC)                    # [b,t2,c,comp,tok]
        zc = (z[:, :, :, 0, :] + 1j * z[:, :, :, 1, :]).astype(np.complex64)
        zc = np.ascontiguousarray(zc.transpose(0, 3, 1, 2)).reshape(
            MESH_BATCH, TC, 8 * P
        )
        tok = slice(c * TC, (c + 1) * TC)
        out[:, tok, :] = zc[:, :, inv]
    return out
